# revision 1
# baseline (speedup 1.0000x reference)
"""Causal self-MQA kernel for Trainium2, sharded over 8 NeuronCores.

Problem: B=2, S=2048, D=2048, H=16 query heads, DH=128, single KV head,
GPT-NeoX RoPE, causal attention, fused q/kv/o projections.

Sharding: 8 cores = 2 batches x 4 head-groups (4 heads = 512 q-dims per
core). The tiny kv projection is replicated within a batch. Each core
computes a partial output [S, D] (its head-group's contribution through
the o-projection); the host sums the 4 partials per batch and adds
o_bias.

Heavy matmuls run as float32r (single-pass PE mode, 4x faster than fp32
for free-dim >= 256). The BIR verifier requires every f32r-matmul
operand's producer to emit f32r, so all matmul-feeding tensors are
declared float32r end-to-end (bit-compatible with fp32 in DRAM).

Layouts keep the feature dim on partitions so no activation transpose is
needed except the tiny V re-layout (16 PE transposes / core):
  qT[dh, s] = wqT.T @ xT          (lhsT = wqT tiles, rhs = xT tiles)
  rotate_half(q) = swap_matrix @ qT   (PE matmul; sign folded into sinT)
  scoresT[k, q] = k_ropeT(dh,k).T @ q_ropeT(dh,q)
  softmax over k = PARTITION dim: no max-subtraction (|scores| < ~7),
    sums via ones-vector matmuls accumulated in PSUM, reciprocal on DVE,
    partition-broadcast on GpSimd.
  attnT[dh, q] += v_nat(k,dh).T @ expT(k,q)   accumulated over k blocks
  out_part[s_blk, d] = attnT_blocks.T @ woT tiles
"""

import os
import sys

import numpy as np

for _p in ("/opt/trn_rl_repo", "/root/.axon_site/_ro/trn_rl_repo"):
    if os.path.isdir(_p) and _p not in sys.path:
        sys.path.insert(0, _p)

import concourse.bass as bass  # noqa: E402,F401
import concourse.mybir as mybir  # noqa: E402
import concourse.tile as tile  # noqa: E402
from concourse import bacc  # noqa: E402
from concourse.bass_utils import run_bass_kernel_spmd  # noqa: E402

B, S, D = 2, 2048, 2048
H, DH = 16, 128
G = 4          # head groups (cores per batch)
HPG = 4        # heads per group
C = HPG * DH   # 512 output dims per group
SC = 256       # projection s-chunk width
NSC = S // SC  # 8
KT = D // 128  # 16 contraction tiles
QC = 512       # attention q-chunk width
NQC = S // QC  # 4
NSB = S // 128  # 16 s-blocks

F32 = mybir.dt.float32
F32R = mybir.dt.float32r
AF = mybir.ActivationFunctionType
OP = mybir.AluOpType

_NC_CACHE = {}


def build_nc():
    nc = bacc.Bacc("TRN2", target_bir_lowering=False, debug=False)

    xT = nc.dram_tensor("xT", [D, S], F32R, kind="ExternalInput").ap()
    wqT = nc.dram_tensor("wqT", [D, C], F32R, kind="ExternalInput").ap()
    wkvT = nc.dram_tensor("wkvT", [D, 2 * DH], F32R, kind="ExternalInput").ap()
    woT = nc.dram_tensor("woT", [C, D], F32R, kind="ExternalInput").ap()
    qb = nc.dram_tensor("qb", [DH, HPG], F32, kind="ExternalInput").ap()
    kvb = nc.dram_tensor("kvb", [DH, 2], F32, kind="ExternalInput").ap()
    cost = nc.dram_tensor("cost", [DH, S], F32, kind="ExternalInput").ap()
    sint = nc.dram_tensor("sint", [DH, S], F32, kind="ExternalInput").ap()
    mask = nc.dram_tensor("mask", [128, 128], F32, kind="ExternalInput").ap()
    mask2 = nc.dram_tensor("mask2", [128, 256], F32, kind="ExternalInput").ap()
    ident = nc.dram_tensor("ident", [128, 128], F32, kind="ExternalInput").ap()
    swap = nc.dram_tensor("swap", [128, 128], F32R, kind="ExternalInput").ap()
    onesk = nc.dram_tensor("onesk", [128, 1], F32R, kind="ExternalInput").ap()
    out_p = nc.dram_tensor("out_p", [S, D], F32, kind="ExternalOutput").ap()

    with tile.TileContext(nc) as tc:
        _body(nc, tc, xT, wqT, wkvT, woT, qb, kvb, cost, sint, mask, mask2,
              ident, swap, onesk, out_p)
    nc.compile()
    return nc


def _body(nc, tc, xT, wqT, wkvT, woT, qb, kvb, cost, sint, mask, mask2,
          ident, swap, onesk, out_p):
    consts = tc.alloc_tile_pool(name="consts", bufs=1)
    sb = tc.alloc_tile_pool(name="sb", bufs=2)
    psum = tc.alloc_tile_pool(name="psum", bufs=1, space="PSUM")

    # ---- constants / weights resident in SBUF (DMAs issued in need-order
    # below, after the first x tiles, so the PE starts early) ----
    cost_sb = consts.tile([DH, S], F32, tag="cost", name="cost")
    sint_sb = consts.tile([DH, S], F32, tag="sint", name="sint")
    mask_sb = consts.tile([128, 128], F32, tag="mask", name="mask")
    mask2_sb = consts.tile([128, 256], F32, tag="mask2", name="mask2")
    ident_sb = consts.tile([128, 128], F32, tag="ident", name="ident")
    swap_sb = consts.tile([128, 128], F32R, tag="swap", name="swap")
    onesk_sb = consts.tile([128, 1], F32R, tag="onesk", name="onesk")
    qb_sb = consts.tile([DH, HPG], F32, tag="qb", name="qb")
    kvb_sb = consts.tile([DH, 2], F32, tag="kvb", name="kvb")
    wq_sb = consts.tile([128, KT, C], F32R, tag="wq", name="wq")
    wkv_sb = consts.tile([128, KT, 2 * DH], F32R, tag="wkv", name="wkv")
    wqT_r = wqT.rearrange("(t p) m -> p t m", p=128)
    wkvT_r = wkvT.rearrange("(t p) m -> p t m", p=128)

    def load_critical_weights():
        # only what the first ~15us of PE work needs, so these DMAs get the
        # full HBM bandwidth: v/k weights + head-0 q weights + small consts.
        # Finely split in consumption order so the first PE matmuls (which
        # need only xt half0 + the t0-7 slice of the v weights) fire early.
        nc.sync.dma_start(qb_sb, qb)
        nc.sync.dma_start(ident_sb, ident)
        nc.sync.dma_start(swap_sb, swap)
        ht = KT // 2
        nc.sync.dma_start(wkv_sb[:, ht:KT, DH:2 * DH],
                          wkvT_r[:, ht:KT, DH:2 * DH])
        for th in range(2):
            tsl = slice(th * ht, (th + 1) * ht)
            nc.sync.dma_start(wkv_sb[:, tsl, 0:DH], wkvT_r[:, tsl, 0:DH])
        for th in range(2):
            tsl = slice(th * ht, (th + 1) * ht)
            nc.sync.dma_start(wq_sb[:, tsl, 0:DH], wqT_r[:, tsl, 0:DH])

    def load_rest_of_weights():
        # issued from the ACT queue AFTER its first v-copy, which keeps
        # these off the HBM port during the critical startup window
        for h in range(1, HPG):
            csl = slice(h * DH, (h + 1) * DH)
            nc.scalar.dma_start(wq_sb[:, :, csl], wqT_r[:, :, csl])
        nc.scalar.dma_start(cost_sb, cost)
        nc.scalar.dma_start(sint_sb, sint)
        nc.scalar.dma_start(mask_sb, mask)
        nc.scalar.dma_start(mask2_sb, mask2)
        nc.scalar.dma_start(onesk_sb, onesk)

    # ---- persistent activations (f32r: they feed f32r matmuls) ----
    q_rope = [consts.tile([DH, S], F32R, tag=f"qrope{h}", name=f"qrope{h}")
              for h in range(HPG)]
    k_rope = consts.tile([DH, S], F32R, tag="krope", name="krope")
    v_nat = consts.tile([128, NSB, DH], F32R, tag="vnat", name="vnat")
    attn = [consts.tile([DH, S], F32R, tag=f"attn{h}", name=f"attn{h}")
            for h in range(HPG)]

    xT_r = xT.rearrange("(t p) s -> p t s", p=128)

    # ================= phase 1: q/kv projections + RoPE =================
    for sc in range(NSC):
        ssl = slice(sc * SC, (sc + 1) * SC)
        xth = []
        for half in range(2):
            xh = sb.tile([128, KT // 2, SC], F32R, tag="xt", name="xt", bufs=4)
            nc.sync.dma_start(
                xh, xT_r[:, half * (KT // 2):(half + 1) * (KT // 2), ssl])
            xth.append(xh)
            if sc == 0 and half == 0:
                nc.sync.dma_start(kvb_sb, kvb)
                nc.sync.dma_start(
                    wkv_sb[:, 0:KT // 2, DH:2 * DH],
                    wkvT_r[:, 0:KT // 2, DH:2 * DH])
        if sc == 0:
            load_critical_weights()

        def xtile(t):
            return xth[t // (KT // 2)][:, t % (KT // 2), :]

        def rope(dst, ps, bias_col):
            """dst[:, ssl] = rope(ps + bias).

            raw = ps + bias (ACT, to SBUF); rot = swap_matrix @ raw (PE);
            dst = raw*cos + rot*sin_signed (DVE x2 + GpSimd add).
            """
            raw = sb.tile([128, SC], F32R, tag="qraw", name="qraw", bufs=4)
            nc.scalar.activation(raw, ps, AF.Identity, bias=bias_col)
            rot = psum.tile([128, SC], F32, tag="score", name="rotps", bufs=4)
            nc.tensor.matmul(rot, swap_sb, raw, start=True, stop=True)
            tmp = sb.tile([128, SC], F32, tag="ropetmp", name="ropetmp",
                          bufs=2)
            nc.vector.tensor_mul(dst[:, ssl], raw, cost_sb[:, ssl])
            nc.vector.tensor_mul(tmp, rot, sint_sb[:, ssl])
            nc.gpsimd.tensor_add(dst[:, ssl], dst[:, ssl], tmp)

        # v first: its ACT-copy + PE-transpose chain overlaps the q matmuls
        ps = psum.tile([128, SC], F32, tag="av", name="proj", bufs=3)
        for t in range(KT):
            nc.tensor.matmul(ps, wkv_sb[:, t, DH:2 * DH], xtile(t),
                             start=(t == 0), stop=(t == KT - 1))
        vt = sb.tile([128, SC], F32, tag="vt", name="vt", bufs=1)
        nc.scalar.activation(vt, ps, AF.Identity, bias=kvb_sb[:, 1:2])
        if sc == 0:
            load_rest_of_weights()

        # k
        ps = psum.tile([128, SC], F32, tag="av", name="proj", bufs=3)
        for t in range(KT):
            nc.tensor.matmul(ps, wkv_sb[:, t, 0:DH], xtile(t),
                             start=(t == 0), stop=(t == KT - 1))
        rope(k_rope, ps, kvb_sb[:, 0:1])

        for h in range(HPG):
            ps = psum.tile([128, SC], F32, tag="av", name="proj", bufs=3)
            for t in range(KT):
                nc.tensor.matmul(
                    ps, wq_sb[:, t, h * DH:(h + 1) * DH], xtile(t),
                    start=(t == 0), stop=(t == KT - 1))
            rope(q_rope[h], ps, qb_sb[:, h:h + 1])
            if h == 0:
                for j in range(SC // 128):
                    tp = psum.tile([128, 128], F32, tag="score", name="tpose",
                                   bufs=4)
                    nc.tensor.transpose(tp, vt[:, j * 128:(j + 1) * 128],
                                        ident_sb)
                    nc.scalar.activation(v_nat[:, sc * (SC // 128) + j, :],
                                         tp, AF.Copy)

    # ====== phases 2+3: causal attention (qc outer, head inner) with the
    # o-projection for q-chunk qc-1 interleaved into qc's attention, so the
    # 16.8 MB of output DMA spreads across the whole attention phase ======
    woT_r = woT.rearrange("(t p) n -> p t n", p=128)
    out_pr = out_p.rearrange("(sb p) n -> p sb n", p=128)

    # wo reuses the (dead after phase 1) wq slot: same tag, same size.
    wo = consts.tile([128, HPG, D], F32R, tag="wq", name="wo")
    nc.sync.dma_start(wo, woT_r)

    def oproj(qc):
        """Project+store output rows of q-chunk qc (4 s-blocks x full D)."""
        for dc in range(4):
            dsl = slice(dc * 512, (dc + 1) * 512)
            for pair in range(2):
                osb = sb.tile([128, 2, 512], F32, tag="osb", name="osb",
                              bufs=2)
                for j in range(2):
                    sblk = qc * 4 + pair * 2 + j
                    op = psum.tile([128, 512], F32, tag="score", name="oproj",
                                   bufs=4)
                    for c in range(HPG):
                        nc.tensor.matmul(
                            op, attn[c][:, sblk * 128:(sblk + 1) * 128],
                            wo[:, c, dsl], start=(c == 0),
                            stop=(c == HPG - 1))
                    if (sblk + dc) % 2 == 0:
                        nc.scalar.activation(osb[:, j, :], op, AF.Copy)
                    else:
                        nc.vector.tensor_copy(osb[:, j, :], op)
                nc.sync.dma_start(
                    out_pr[:, qc * 4 + pair * 2:qc * 4 + pair * 2 + 2, dsl],
                    osb)

    # Flat 2-deep pipeline over ALL (qc, h, kj) regions — no drain at unit
    # boundaries; oproj(qc-1) slots in after the (h==1, qc) unit finishes.
    units = [(h, qc) for qc in range(NQC) for h in range(HPG)]
    seq = []
    for ui, (h, qc) in enumerate(units):
        for kj in range(4 * qc + 4):
            seq.append((ui, kj))
    ustate = {}

    def emit_scores(ui, kj):
        h, qc = units[ui]
        st = max(0, kj * 128 - qc * QC)
        wide_diag = st == 384  # widen to 256 cols: f32r needs N>=256 for
        if wide_diag:          # full rate; the extra 128 cols get -1e9
            st = 256
        width = QC - st
        sp = psum.tile([128, QC], F32, tag="score", name="score", bufs=4)
        nc.tensor.matmul(
            sp[:, 0:width],
            k_rope[:, kj * 128:(kj + 1) * 128],
            q_rope[h][:, qc * QC + st:(qc + 1) * QC],
            start=True, stop=True)
        if wide_diag:
            nc.vector.tensor_add(sp[:, 0:256], sp[:, 0:256], mask2_sb)
        elif kj >= 4 * qc:  # region starts at the diagonal block
            nc.vector.tensor_add(sp[:, 0:128], sp[:, 0:128], mask_sb)
        et = sb.tile([128, QC], F32R, tag="exp", name="exp", bufs=4)
        nc.scalar.activation(et[:, 0:width], sp[:, 0:width], AF.Exp)
        return et, st, width

    def emit_av(ui, kj, ready):
        h, qc = units[ui]
        et, st, width = ready
        if kj == 0:
            ustate[ui] = (
                psum.tile([128, QC], F32, tag="av", name="av", bufs=3),
                psum.tile([1, QC], F32, tag="sums", name="sums", bufs=1),
            )
        att_ps, sum_ps = ustate[ui]
        last = kj == 4 * qc + 3
        nc.tensor.matmul(
            att_ps[:, st:QC], v_nat[:, kj, :], et[:, 0:width],
            start=(kj == 0), stop=last, skip_group_check=True)
        nc.tensor.matmul(
            sum_ps[0:1, st:QC], onesk_sb, et[:, 0:width],
            start=(kj == 0), stop=last, skip_group_check=True)
        if last:
            rec = sb.tile([1, QC], F32, tag="rec", name="rec", bufs=1)
            nc.vector.reciprocal(rec, sum_ps[0:1, :])
            bcs = sb.tile([128, QC], F32, tag="bcs", name="bcs", bufs=2)
            nc.gpsimd.partition_broadcast(bcs, rec, channels=128)
            nc.vector.tensor_mul(attn[h][:, qc * QC:(qc + 1) * QC],
                                 att_ps, bcs)
            del ustate[ui]
            if h == 1 and qc > 0:
                oproj(qc - 1)

    LOOKAHEAD = 3
    ready = {}
    for i in range(min(LOOKAHEAD, len(seq))):
        ready[i] = emit_scores(*seq[i])
    for i in range(len(seq)):
        nxt = i + LOOKAHEAD
        if nxt < len(seq):
            ready[nxt] = emit_scores(*seq[nxt])
        emit_av(*seq[i], ready.pop(i))
    oproj(NQC - 1)

    psum.release()
    sb.release()
    consts.release()


def _host_tables():
    c4 = np.float32(1.0) / np.sqrt(np.sqrt(np.float32(DH)))
    inv_freq = (np.float32(1.0) / np.power(
        np.float32(10000.0),
        np.arange(0, DH, 2, dtype=np.float32) / np.float32(DH))).astype(np.float32)
    t = np.arange(S, dtype=np.float32)
    freqs = np.outer(t, inv_freq).astype(np.float32)          # [S, 64]
    emb = np.concatenate([freqs, freqs], axis=1)              # [S, 128]
    cost = (np.cos(emb).T * c4).astype(np.float32).copy()     # [128, S]
    sint = (np.sin(emb).T * c4).astype(np.float32)
    sint[0:64] *= np.float32(-1.0)                            # rotate_half sign
    sint = sint.astype(np.float32).copy()
    kq = np.arange(128, dtype=np.int64)
    mask = np.where(kq[None, :] >= kq[:, None], np.float32(0.0),
                    np.float32(-1e9)).astype(np.float32)      # [k,q]
    mask2 = np.full((128, 256), np.float32(-1e9), np.float32)
    mask2[:, 128:] = mask
    ident = np.eye(128, dtype=np.float32)
    # swap[i, j] = 1 iff j == (i+64) % 128; symmetric, so it works as lhsT.
    swap = np.zeros((128, 128), np.float32)
    swap[kq, (kq + 64) % 128] = np.float32(1.0)
    return cost, sint, mask, mask2, ident, swap


def kernel(x, q_weight, q_bias, kv_weight, kv_bias, o_weight, o_bias):
    x = np.asarray(x, np.float32)
    q_weight = np.asarray(q_weight, np.float32)
    q_bias = np.asarray(q_bias, np.float32)
    kv_weight = np.asarray(kv_weight, np.float32)
    kv_bias = np.asarray(kv_bias, np.float32)
    o_weight = np.asarray(o_weight, np.float32)
    o_bias = np.asarray(o_bias, np.float32)

    if "nc" not in _NC_CACHE:
        _NC_CACHE["nc"] = build_nc()
    nc = _NC_CACHE["nc"]

    cost, sint, mask, mask2, ident, swap = _host_tables()
    onesk = np.ones((128, 1), np.float32)
    kvb = kv_bias.reshape(2, DH).T.copy()

    xTs = [np.ascontiguousarray(x[b].T) for b in range(B)]
    wkvT = np.ascontiguousarray(kv_weight.T)

    in_maps = []
    for core in range(8):
        b, g = divmod(core, G)
        in_maps.append({
            "xT": xTs[b],
            "wqT": np.ascontiguousarray(q_weight[g * C:(g + 1) * C].T),
            "wkvT": wkvT,
            "woT": np.ascontiguousarray(o_weight[:, g * C:(g + 1) * C].T),
            "qb": np.ascontiguousarray(q_bias[g * C:(g + 1) * C].reshape(HPG, DH).T),
            "kvb": kvb,
            "cost": cost,
            "sint": sint,
            "mask": mask,
            "mask2": mask2,
            "ident": ident,
            "swap": swap,
            "onesk": onesk,
        })

    res = run_bass_kernel_spmd(nc, in_maps, core_ids=list(range(8)))

    out = np.zeros((B, S, D), np.float32)
    for core in range(8):
        out[core // G] += res.results[core]["out_p"]
    out += o_bias[None, None, :]
    return out



# revision 10
# speedup vs baseline: 1.0634x; 1.0634x over previous
"""Causal self-MQA kernel for Trainium2, sharded over 8 NeuronCores.

Problem: B=2, S=2048, D=2048, H=16 query heads, DH=128, single KV head,
GPT-NeoX RoPE, causal attention, fused q/kv/o projections.

Sharding: 8 cores = 2 batches x 4 head-groups (4 heads = 512 q-dims per
core). The tiny kv projection is replicated within a batch. Each core
computes a partial output [S, D] (its head-group's contribution through
the o-projection); the host sums the 4 partials per batch and adds
o_bias.

All matmul operands are bf16 (PSUM accumulation stays fp32): same PE
rate as f32r but half the DMA/SBUF footprint, which removes the
DMA-bound phase-1 head, and no N>=256 restriction so the causal
diagonal needs no widening. End-to-end rel err ~5e-3 vs the 2e-2 gate.

Layouts keep the feature dim on partitions so no activation transpose is
needed except the tiny V re-layout (16 PE transposes / core):
  qT[dh, s] = wqT.T @ xT          (lhsT = wqT tiles, rhs = xT tiles)
  rotate_half(q) = swap_matrix @ qT   (PE matmul; sign folded into sinT)
  scoresT[k, q] = k_ropeT(dh,k).T @ q_ropeT(dh,q)
  softmax sums over k (= partition dim) via TRANSPOSED ones-matmuls:
    sums[q, 1] += et[k, q-block].T @ ones[k, 1] -- moving dim is 1, so
    the PE cost is ~nil (vs streaming et a second time).  The per-unit
    [q, 4] sums column is reciprocal'd, PE-transposed to a row, and
    partition-broadcast for the normalizing multiply.
  attnT[dh, q] += v_nat(k,dh).T @ expT(k,q)   accumulated over k blocks
  out_part[s_blk, d] = attnT_blocks.T @ woT tiles, with the o-projection
    slots spread between attention regions so output DMA never bursts.
"""

import os
import sys

import ml_dtypes
import numpy as np

for _p in ("/opt/trn_rl_repo", "/root/.axon_site/_ro/trn_rl_repo"):
    if os.path.isdir(_p) and _p not in sys.path:
        sys.path.insert(0, _p)

import concourse.bass as bass  # noqa: E402,F401
import concourse.mybir as mybir  # noqa: E402
import concourse.tile as tile  # noqa: E402
from concourse import bacc  # noqa: E402
from concourse.bass_utils import run_bass_kernel_spmd  # noqa: E402

B, S, D = 2, 2048, 2048
H, DH = 16, 128
G = 4          # head groups (cores per batch)
HPG = 4        # heads per group
C = HPG * DH   # 512 output dims per group
SC = 256       # projection s-chunk width
NSC = S // SC  # 8
KT = D // 128  # 16 contraction tiles
QC = 512       # attention q-chunk width
NQC = S // QC  # 4
NSB = S // 128  # 16 s-blocks

F32 = mybir.dt.float32
BF16 = mybir.dt.bfloat16
AF = mybir.ActivationFunctionType
OP = mybir.AluOpType
NPBF16 = ml_dtypes.bfloat16

_NC_CACHE = {}


def build_nc():
    nc = bacc.Bacc("TRN2", target_bir_lowering=False, debug=False)

    xT = nc.dram_tensor("xT", [D, S], BF16, kind="ExternalInput").ap()
    wqT = nc.dram_tensor("wqT", [D, C], BF16, kind="ExternalInput").ap()
    wkvT = nc.dram_tensor("wkvT", [D, 2 * DH], BF16, kind="ExternalInput").ap()
    woT = nc.dram_tensor("woT", [C, D], BF16, kind="ExternalInput").ap()
    qb = nc.dram_tensor("qb", [DH, HPG], F32, kind="ExternalInput").ap()
    kvb = nc.dram_tensor("kvb", [DH, 2], F32, kind="ExternalInput").ap()
    cost = nc.dram_tensor("cost", [DH, S], BF16, kind="ExternalInput").ap()
    sint = nc.dram_tensor("sint", [DH, S], BF16, kind="ExternalInput").ap()
    mask = nc.dram_tensor("mask", [128, 128], F32, kind="ExternalInput").ap()
    ident = nc.dram_tensor("ident", [128, 128], BF16, kind="ExternalInput").ap()
    swap = nc.dram_tensor("swap", [128, 128], BF16, kind="ExternalInput").ap()
    onesk = nc.dram_tensor("onesk", [128, 1], BF16, kind="ExternalInput").ap()
    out_p = nc.dram_tensor("out_p", [S, D], BF16, kind="ExternalOutput").ap()

    with tile.TileContext(nc) as tc:
        _body(nc, tc, xT, wqT, wkvT, woT, qb, kvb, cost, sint, mask,
              ident, swap, onesk, out_p)
    nc.compile()
    return nc


def _body(nc, tc, xT, wqT, wkvT, woT, qb, kvb, cost, sint, mask,
          ident, swap, onesk, out_p):
    consts = tc.alloc_tile_pool(name="consts", bufs=1)
    sb = tc.alloc_tile_pool(name="sb", bufs=2)
    psum = tc.alloc_tile_pool(name="psum", bufs=1, space="PSUM")

    cost_sb = consts.tile([DH, S], BF16, tag="cost", name="cost")
    sint_sb = consts.tile([DH, S], BF16, tag="sint", name="sint")
    mask_sb = consts.tile([128, 128], F32, tag="mask", name="mask")
    ident_sb = consts.tile([128, 128], BF16, tag="ident", name="ident")
    swap_sb = consts.tile([128, 128], BF16, tag="swap", name="swap")
    onesk_sb = consts.tile([128, 1], BF16, tag="onesk", name="onesk")
    qb_sb = consts.tile([DH, HPG], F32, tag="qb", name="qb")
    kvb_sb = consts.tile([DH, 2], F32, tag="kvb", name="kvb")
    wq_sb = consts.tile([128, KT, C], BF16, tag="wq", name="wq")
    wkv_sb = consts.tile([128, KT, 2 * DH], BF16, tag="wkv", name="wkv")
    wqT_r = wqT.rearrange("(t p) m -> p t m", p=128)
    wkvT_r = wkvT.rearrange("(t p) m -> p t m", p=128)

    def load_rest_of_weights():
        # issued from the ACT queue AFTER its first v-copy, behind the
        # sync-queue startup stream at the DMA engines
        for h in range(1, HPG):
            csl = slice(h * DH, (h + 1) * DH)
            nc.scalar.dma_start(wq_sb[:, :, csl], wqT_r[:, :, csl])
        nc.scalar.dma_start(cost_sb, cost)
        nc.scalar.dma_start(sint_sb, sint)
        nc.scalar.dma_start(mask_sb, mask)
        nc.scalar.dma_start(onesk_sb, onesk)

    # ---- persistent activations (bf16: they feed bf16 matmuls) ----
    q_rope = [consts.tile([DH, S], BF16, tag=f"qrope{h}", name=f"qrope{h}")
              for h in range(HPG)]
    k_rope = consts.tile([DH, S], BF16, tag="krope", name="krope")
    v_nat = consts.tile([128, NSB, DH], BF16, tag="vnat", name="vnat")
    attn = [consts.tile([DH, S], BF16, tag=f"attn{h}", name=f"attn{h}")
            for h in range(HPG)]

    xT_r = xT.rearrange("(t p) s -> p t s", p=128)

    # ================= phase 1: q/kv projections + RoPE =================
    for sc in range(NSC):
        ssl = slice(sc * SC, (sc + 1) * SC)
        if sc == 0:
            # fine-grained, consumption-ordered startup stream: the first
            # v matmuls need only wkv_v[t0-3] + x[t0-1], so those land
            # first; everything else follows in PE need-order.
            xh0 = sb.tile([128, KT // 2, SC], BF16, tag="xt", name="xt",
                          bufs=6)
            xh1 = sb.tile([128, KT // 2, SC], BF16, tag="xt", name="xt",
                          bufs=6)
            nc.sync.dma_start(wkv_sb[:, 0:4, DH:2 * DH],
                              wkvT_r[:, 0:4, DH:2 * DH])
            nc.sync.dma_start(xh0[:, 0:2, :], xT_r[:, 0:2, ssl])
            nc.sync.dma_start(xh0[:, 2:4, :], xT_r[:, 2:4, ssl])
            nc.sync.dma_start(wkv_sb[:, 4:8, DH:2 * DH],
                              wkvT_r[:, 4:8, DH:2 * DH])
            nc.sync.dma_start(xh0[:, 4:8, :], xT_r[:, 4:8, ssl])
            nc.sync.dma_start(wkv_sb[:, 8:KT, DH:2 * DH],
                              wkvT_r[:, 8:KT, DH:2 * DH])
            nc.sync.dma_start(xh1[:, 0:4, :], xT_r[:, 8:12, ssl])
            nc.sync.dma_start(xh1[:, 4:8, :], xT_r[:, 12:16, ssl])
            nc.sync.dma_start(kvb_sb, kvb)
            for th in range(2):
                tsl = slice(th * 8, (th + 1) * 8)
                nc.sync.dma_start(wkv_sb[:, tsl, 0:DH], wkvT_r[:, tsl, 0:DH])
            for th in range(2):
                tsl = slice(th * 8, (th + 1) * 8)
                nc.sync.dma_start(wq_sb[:, tsl, 0:DH], wqT_r[:, tsl, 0:DH])
            nc.sync.dma_start(qb_sb, qb)
            nc.sync.dma_start(swap_sb, swap)
            nc.sync.dma_start(ident_sb, ident)
            xth = [xh0, xh1]
        else:
            xth = []
            for half in range(2):
                xh = sb.tile([128, KT // 2, SC], BF16, tag="xt", name="xt",
                             bufs=6)
                nc.sync.dma_start(
                    xh, xT_r[:, half * (KT // 2):(half + 1) * (KT // 2), ssl])
                xth.append(xh)

        def xtile(t):
            return xth[t // (KT // 2)][:, t % (KT // 2), :]

        def rope(dst, ps, bias_col):
            """dst[:, ssl] = rope(ps + bias).

            raw = ps + bias (ACT, to SBUF); rot = swap_matrix @ raw (PE);
            dst = raw*cos + rot*sin_signed (DVE x2 + GpSimd add).
            """
            raw = sb.tile([128, SC], BF16, tag="qraw", name="qraw", bufs=4)
            nc.scalar.activation(raw, ps, AF.Identity, bias=bias_col)
            rot = psum.tile([128, SC], F32, tag="score", name="rotps", bufs=3)
            nc.tensor.matmul(rot, swap_sb, raw, start=True, stop=True)
            tmp = sb.tile([128, SC], BF16, tag="ropetmp", name="ropetmp",
                          bufs=2)
            nc.vector.tensor_mul(dst[:, ssl], raw, cost_sb[:, ssl])
            nc.vector.tensor_mul(tmp, rot, sint_sb[:, ssl])
            nc.gpsimd.tensor_add(dst[:, ssl], dst[:, ssl], tmp)

        # v first: its ACT-copy + PE-transpose chain overlaps the q matmuls
        ps = psum.tile([128, SC], F32, tag="av", name="proj", bufs=2)
        for t in range(KT):
            nc.tensor.matmul(ps, wkv_sb[:, t, DH:2 * DH], xtile(t),
                             start=(t == 0), stop=(t == KT - 1))
        vt = sb.tile([128, SC], BF16, tag="vt", name="vt", bufs=1)
        nc.scalar.activation(vt, ps, AF.Identity, bias=kvb_sb[:, 1:2])
        if sc == 0:
            load_rest_of_weights()

        # k
        ps = psum.tile([128, SC], F32, tag="av", name="proj", bufs=2)
        for t in range(KT):
            nc.tensor.matmul(ps, wkv_sb[:, t, 0:DH], xtile(t),
                             start=(t == 0), stop=(t == KT - 1))
        rope(k_rope, ps, kvb_sb[:, 0:1])

        for h in range(HPG):
            ps = psum.tile([128, SC], F32, tag="av", name="proj", bufs=2)
            for t in range(KT):
                nc.tensor.matmul(
                    ps, wq_sb[:, t, h * DH:(h + 1) * DH], xtile(t),
                    start=(t == 0), stop=(t == KT - 1))
            rope(q_rope[h], ps, qb_sb[:, h:h + 1])
            if h == 0:
                for j in range(SC // 128):
                    tp = psum.tile([128, 128], BF16, tag="tp", name="tpose",
                                   bufs=1)
                    nc.tensor.transpose(tp, vt[:, j * 128:(j + 1) * 128],
                                        ident_sb)
                    nc.scalar.activation(v_nat[:, sc * (SC // 128) + j, :],
                                         tp, AF.Copy)

    # ====== phases 2+3: causal attention with the o-projection slots
    # spread between attention regions, so the output DMA and PSUM-copy
    # work never bursts enough to stall the PE ======
    woT_r = woT.rearrange("(t p) n -> p t n", p=128)
    out_pr = out_p.rearrange("(sb p) n -> p sb n", p=128)

    # wo reuses the (dead after phase 1) wq slot: same tag, same size.
    wo = consts.tile([128, HPG, D], BF16, tag="wq", name="wo")
    for dc in range(4):
        dsl = slice(dc * 512, (dc + 1) * 512)
        nc.sync.dma_start(wo[:, :, dsl], woT_r[:, :, dsl])

    def emit_oproj_slot(qc, dc, pair):
        """Project+store 2 s-blocks x 512 d-cols of q-chunk qc."""
        dsl = slice(dc * 512, (dc + 1) * 512)
        osb = sb.tile([128, 2, 512], BF16, tag="osb", name="osb", bufs=4)
        for j in range(2):
            sblk = qc * 4 + pair * 2 + j
            op = psum.tile([128, 512], F32, tag="score", name="oproj",
                           bufs=3)
            for c in range(HPG):
                nc.tensor.matmul(
                    op, attn[c][:, sblk * 128:(sblk + 1) * 128],
                    wo[:, c, dsl], start=(c == 0), stop=(c == HPG - 1))
            if (sblk + dc) % 2 == 0:
                nc.scalar.activation(osb[:, j, :], op, AF.Copy)
            else:
                nc.vector.tensor_copy(osb[:, j, :], op)
        nc.sync.dma_start(
            out_pr[:, qc * 4 + pair * 2:qc * 4 + pair * 2 + 2, dsl], osb)

    # Unit order interleaves qc=0 with qc=1 so the tiny qc=0 units (whose
    # normalization tail is longer than their PE time) hide inside big
    # neighbours; oproj(qc) slots then spread into the following units.
    units = ([(h, qc) for h in range(HPG) for qc in (0, 1)]
             + [(h, 2) for h in range(HPG)] + [(h, 3) for h in range(HPG)])
    seq = []
    for ui, (h, qc) in enumerate(units):
        for kj in range(4 * qc + 4):
            seq.append((ui, kj))
    ustate = {}
    done_h = {qc: 0 for qc in range(NQC)}

    def emit_scores(ui, kj):
        h, qc = units[ui]
        st = max(0, kj * 128 - qc * QC)
        width = QC - st
        sp = psum.tile([128, QC], F32, tag="score", name="score", bufs=3)
        nc.tensor.matmul(
            sp[:, 0:width],
            k_rope[:, kj * 128:(kj + 1) * 128],
            q_rope[h][:, qc * QC + st:(qc + 1) * QC],
            start=True, stop=True)
        if kj >= 4 * qc:  # region starts at the diagonal block
            nc.vector.tensor_add(sp[:, 0:128], sp[:, 0:128], mask_sb)
        et = sb.tile([128, QC], BF16, tag="exp", name="exp", bufs=4)
        nc.scalar.activation(et[:, 0:width], sp[:, 0:width], AF.Exp)
        return et, st, width

    def emit_av(ui, kj, ready):
        """Returns qc when this region completes all units of chunk qc."""
        h, qc = units[ui]
        et, st, width = ready
        if kj == 0:
            ustate[ui] = (
                psum.tile([128, QC], F32, tag="av", name="av", bufs=2),
                psum.tile([128, 4], F32, tag="sums", name="sums", bufs=2),
            )
        att_ps, sum_ps = ustate[ui]
        last = kj == 4 * qc + 3
        nc.tensor.matmul(
            att_ps[:, st:QC], v_nat[:, kj, :], et[:, 0:width],
            start=(kj == 0), stop=last, skip_group_check=True)
        # softmax sums, transposed: stationary = et q-block, moving = ones
        # -> out [q, 1]; PE cost ~1 cycle instead of `width`.  All 16ish
        # matmuls form ONE accumulation group on the bank: start only on
        # the very first (zeroes the whole bank buffer), stop only on the
        # very last (commits) -- an extra start would wipe pending sums.
        for qb in range(st // 128, 4):
            nc.tensor.matmul(
                sum_ps[:, qb:qb + 1],
                et[:, qb * 128 - st:qb * 128 - st + 128],
                onesk_sb,
                start=(kj == 0 and qb == 0),
                stop=(last and qb == 3),
                skip_group_check=True)
        if not last:
            return None
        rtmp = sb.tile([128, 4], BF16, tag="rtmp", name="rtmp", bufs=2)
        with nc.allow_low_precision(reason="bf16 softmax denom, ~0.4% ok"):
            nc.vector.reciprocal(rtmp, sum_ps)
        # transpose each 128-long reciprocal column to a [1, 128] row at
        # partition 0 (partition_broadcast requires a partition-0 source)
        tp = psum.tile([1, QC], BF16, tag="tp", name="rectp", bufs=1)
        for qb in range(4):
            nc.tensor.matmul(tp[0:1, qb * 128:(qb + 1) * 128],
                             rtmp[:, qb:qb + 1], ident_sb,
                             is_transpose=True,
                             start=(qb == 0), stop=(qb == 3),
                             skip_group_check=True)
        rrow = sb.tile([1, QC], BF16, tag="rrow", name="rrow", bufs=2)
        nc.scalar.activation(rrow, tp, AF.Copy)
        bcs = sb.tile([128, QC], BF16, tag="bcs", name="bcs", bufs=2)
        nc.gpsimd.partition_broadcast(bcs, rrow, channels=128)
        nc.vector.tensor_mul(attn[h][:, qc * QC:(qc + 1) * QC], att_ps, bcs)
        del ustate[ui]
        done_h[qc] += 1
        return qc if done_h[qc] == HPG else None

    pending = []
    LOOKAHEAD = 3
    ready = {}
    for i in range(min(LOOKAHEAD, len(seq))):
        ready[i] = emit_scores(*seq[i])
    for i in range(len(seq)):
        nxt = i + LOOKAHEAD
        if nxt < len(seq):
            ready[nxt] = emit_scores(*seq[nxt])
        fin = emit_av(*seq[i], ready.pop(i))
        if fin is not None:
            pending.extend(
                (fin, dc, pair) for dc in range(4) for pair in range(2))
        if pending and i % 2 == 0:
            emit_oproj_slot(*pending.pop(0))
    while pending:
        emit_oproj_slot(*pending.pop(0))

    psum.release()
    sb.release()
    consts.release()


def _host_tables():
    c4 = np.float32(1.0) / np.sqrt(np.sqrt(np.float32(DH)))
    inv_freq = (np.float32(1.0) / np.power(
        np.float32(10000.0),
        np.arange(0, DH, 2, dtype=np.float32) / np.float32(DH))).astype(np.float32)
    t = np.arange(S, dtype=np.float32)
    freqs = np.outer(t, inv_freq).astype(np.float32)          # [S, 64]
    emb = np.concatenate([freqs, freqs], axis=1)              # [S, 128]
    cost = (np.cos(emb).T * c4).astype(NPBF16).copy()         # [128, S]
    sint = (np.sin(emb).T * c4).astype(np.float32)
    sint[0:64] *= np.float32(-1.0)                            # rotate_half sign
    sint = sint.astype(NPBF16).copy()
    kq = np.arange(128, dtype=np.int64)
    mask = np.where(kq[None, :] >= kq[:, None], np.float32(0.0),
                    np.float32(-1e9)).astype(np.float32)      # [k,q]
    ident = np.eye(128, dtype=np.float32).astype(NPBF16)
    # swap[i, j] = 1 iff j == (i+64) % 128; symmetric, so it works as lhsT.
    swap = np.zeros((128, 128), np.float32)
    swap[kq, (kq + 64) % 128] = np.float32(1.0)
    return cost, sint, mask, ident, swap.astype(NPBF16)


def kernel(x, q_weight, q_bias, kv_weight, kv_bias, o_weight, o_bias):
    x = np.asarray(x, np.float32)
    q_weight = np.asarray(q_weight, np.float32)
    q_bias = np.asarray(q_bias, np.float32)
    kv_weight = np.asarray(kv_weight, np.float32)
    kv_bias = np.asarray(kv_bias, np.float32)
    o_weight = np.asarray(o_weight, np.float32)
    o_bias = np.asarray(o_bias, np.float32)

    if "nc" not in _NC_CACHE:
        _NC_CACHE["nc"] = build_nc()
    nc = _NC_CACHE["nc"]

    cost, sint, mask, ident, swap = _host_tables()
    onesk = np.ones((128, 1), NPBF16)
    kvb = kv_bias.reshape(2, DH).T.copy()

    xTs = [np.ascontiguousarray(x[b].T).astype(NPBF16) for b in range(B)]
    wkvT = np.ascontiguousarray(kv_weight.T).astype(NPBF16)

    in_maps = []
    for core in range(8):
        b, g = divmod(core, G)
        in_maps.append({
            "xT": xTs[b],
            "wqT": np.ascontiguousarray(
                q_weight[g * C:(g + 1) * C].T).astype(NPBF16),
            "wkvT": wkvT,
            "woT": np.ascontiguousarray(
                o_weight[:, g * C:(g + 1) * C].T).astype(NPBF16),
            "qb": np.ascontiguousarray(
                q_bias[g * C:(g + 1) * C].reshape(HPG, DH).T),
            "kvb": kvb,
            "cost": cost,
            "sint": sint,
            "mask": mask,
            "ident": ident,
            "swap": swap,
            "onesk": onesk,
        })

    res = run_bass_kernel_spmd(nc, in_maps, core_ids=list(range(8)))

    out = np.zeros((B, S, D), np.float32)
    for core in range(8):
        out[core // G] += np.asarray(res.results[core]["out_p"],
                                     dtype=np.float32)
    out += o_bias[None, None, :]
    return out


# revision 14
# speedup vs baseline: 1.0689x; 1.0052x over previous
"""Causal self-MQA kernel for Trainium2, sharded over 8 NeuronCores.

Problem: B=2, S=2048, D=2048, H=16 query heads, DH=128, single KV head,
GPT-NeoX RoPE, causal attention, fused q/kv/o projections.

Sharding: 8 cores = 2 batches x 4 head-groups (4 heads = 512 q-dims per
core). The tiny kv projection is replicated within a batch. Each core
computes a partial output [S, D] (its head-group's contribution through
the o-projection); the host sums the 4 partials per batch and adds
o_bias.

All matmul operands are bf16 (PSUM accumulation stays fp32): same PE
rate as f32r but half the DMA/SBUF footprint, which removes the
DMA-bound phase-1 head, and no N>=256 restriction so the causal
diagonal needs no widening. End-to-end rel err ~5e-3 vs the 2e-2 gate.

Layouts keep the feature dim on partitions so no activation transpose is
needed except the tiny V re-layout (16 PE transposes / core):
  qT[dh, s] = wqT.T @ xT          (lhsT = wqT tiles, rhs = xT tiles)
  rotate_half(q) = swap_matrix @ qT   (PE matmul; sign folded into sinT)
  scoresT[k, q] = k_ropeT(dh,k).T @ q_ropeT(dh,q)
  softmax sums over k (= partition dim) via TRANSPOSED ones-matmuls:
    sums[q, 1] += et[k, q-block].T @ ones[k, 1] -- moving dim is 1, so
    the PE cost is ~nil (vs streaming et a second time).  The per-unit
    [q, 4] sums column is reciprocal'd, PE-transposed to a row, and
    partition-broadcast for the normalizing multiply.
  attnT[dh, q] += v_nat(k,dh).T @ expT(k,q)   accumulated over k blocks
  out_part[s_blk, d] = attnT_blocks.T @ woT tiles, with the o-projection
    slots spread between attention regions so output DMA never bursts.
"""

import os
import sys

import ml_dtypes
import numpy as np

for _p in ("/opt/trn_rl_repo", "/root/.axon_site/_ro/trn_rl_repo"):
    if os.path.isdir(_p) and _p not in sys.path:
        sys.path.insert(0, _p)

import concourse.bass as bass  # noqa: E402,F401
import concourse.mybir as mybir  # noqa: E402
import concourse.tile as tile  # noqa: E402
from concourse import bacc  # noqa: E402
from concourse.bass_utils import run_bass_kernel_spmd  # noqa: E402

B, S, D = 2, 2048, 2048
H, DH = 16, 128
G = 4          # head groups (cores per batch)
HPG = 4        # heads per group
C = HPG * DH   # 512 output dims per group
SC = 256       # projection s-chunk width
NSC = S // SC  # 8
KT = D // 128  # 16 contraction tiles
QC = 512       # attention q-chunk width
NQC = S // QC  # 4
NSB = S // 128  # 16 s-blocks

F32 = mybir.dt.float32
BF16 = mybir.dt.bfloat16
AF = mybir.ActivationFunctionType
OP = mybir.AluOpType
NPBF16 = ml_dtypes.bfloat16

_NC_CACHE = {}


def build_nc():
    nc = bacc.Bacc("TRN2", target_bir_lowering=False, debug=False)

    xT = nc.dram_tensor("xT", [D, S], BF16, kind="ExternalInput").ap()
    wqT = nc.dram_tensor("wqT", [D, C], BF16, kind="ExternalInput").ap()
    wkvT = nc.dram_tensor("wkvT", [D, 2 * DH], BF16, kind="ExternalInput").ap()
    woT = nc.dram_tensor("woT", [C, D], BF16, kind="ExternalInput").ap()
    qb = nc.dram_tensor("qb", [DH, HPG], F32, kind="ExternalInput").ap()
    kvb = nc.dram_tensor("kvb", [DH, 2], F32, kind="ExternalInput").ap()
    cost = nc.dram_tensor("cost", [DH, S], BF16, kind="ExternalInput").ap()
    sint = nc.dram_tensor("sint", [DH, S], BF16, kind="ExternalInput").ap()
    mask = nc.dram_tensor("mask", [128, 128], F32, kind="ExternalInput").ap()
    ident = nc.dram_tensor("ident", [128, 128], BF16, kind="ExternalInput").ap()
    swap = nc.dram_tensor("swap", [128, 128], BF16, kind="ExternalInput").ap()
    onesk = nc.dram_tensor("onesk", [128, 1], BF16, kind="ExternalInput").ap()
    out_p = nc.dram_tensor("out_p", [S, D], BF16, kind="ExternalOutput").ap()

    with tile.TileContext(nc) as tc:
        _body(nc, tc, xT, wqT, wkvT, woT, qb, kvb, cost, sint, mask,
              ident, swap, onesk, out_p)
    nc.compile()
    return nc


def _body(nc, tc, xT, wqT, wkvT, woT, qb, kvb, cost, sint, mask,
          ident, swap, onesk, out_p):
    consts = tc.alloc_tile_pool(name="consts", bufs=1)
    sb = tc.alloc_tile_pool(name="sb", bufs=2)
    psum = tc.alloc_tile_pool(name="psum", bufs=1, space="PSUM")

    cost_sb = consts.tile([DH, S], BF16, tag="cost", name="cost")
    sint_sb = consts.tile([DH, S], BF16, tag="sint", name="sint")
    mask_sb = consts.tile([128, 128], F32, tag="mask", name="mask")
    ident_sb = consts.tile([128, 128], BF16, tag="ident", name="ident")
    swap_sb = consts.tile([128, 128], BF16, tag="swap", name="swap")
    onesk_sb = consts.tile([128, 1], BF16, tag="onesk", name="onesk")
    qb_sb = consts.tile([DH, HPG], F32, tag="qb", name="qb")
    kvb_sb = consts.tile([DH, 2], F32, tag="kvb", name="kvb")
    wq_sb = consts.tile([128, KT, C], BF16, tag="wq", name="wq")
    wkv_sb = consts.tile([128, KT, 2 * DH], BF16, tag="wkv", name="wkv")
    wqT_r = wqT.rearrange("(t p) m -> p t m", p=128)
    wkvT_r = wkvT.rearrange("(t p) m -> p t m", p=128)

    def load_rest_of_weights():
        # issued from the ACT queue AFTER its first v-copy, behind the
        # sync-queue startup stream at the DMA engines
        for h in range(1, HPG):
            csl = slice(h * DH, (h + 1) * DH)
            nc.scalar.dma_start(wq_sb[:, :, csl], wqT_r[:, :, csl])
        nc.scalar.dma_start(cost_sb, cost)
        nc.scalar.dma_start(sint_sb, sint)
        nc.scalar.dma_start(mask_sb, mask)
        nc.scalar.dma_start(onesk_sb, onesk)

    # ---- persistent activations (bf16: they feed bf16 matmuls) ----
    q_rope = [consts.tile([DH, S], BF16, tag=f"qrope{h}", name=f"qrope{h}")
              for h in range(HPG)]
    k_rope = consts.tile([DH, S], BF16, tag="krope", name="krope")
    v_nat = consts.tile([128, NSB, DH], BF16, tag="vnat", name="vnat")
    attn = [consts.tile([DH, S], BF16, tag=f"attn{h}", name=f"attn{h}")
            for h in range(HPG)]

    xT_r = xT.rearrange("(t p) s -> p t s", p=128)

    # ================= phase 1: q/kv projections + RoPE =================
    for sc in range(NSC):
        ssl = slice(sc * SC, (sc + 1) * SC)
        if sc == 0:
            # fine-grained, consumption-ordered startup stream: the first
            # v matmuls need only wkv_v[t0-3] + x[t0-1], so those land
            # first; everything else follows in PE need-order.
            xh0 = sb.tile([128, KT // 2, SC], BF16, tag="xt", name="xt",
                          bufs=6)
            xh1 = sb.tile([128, KT // 2, SC], BF16, tag="xt", name="xt",
                          bufs=6)
            # parallel queues for the first two pieces (HWDGE gens overlap)
            nc.sync.dma_start(wkv_sb[:, :, DH:2 * DH],
                              wkvT_r[:, :, DH:2 * DH])
            nc.scalar.dma_start(xh0, xT_r[:, 0:8, ssl])
            nc.sync.dma_start(xh1, xT_r[:, 8:16, ssl])
            nc.sync.dma_start(kvb_sb, kvb)
            nc.sync.dma_start(wkv_sb[:, :, 0:DH], wkvT_r[:, :, 0:DH])
            nc.sync.dma_start(wq_sb[:, :, 0:DH], wqT_r[:, :, 0:DH])
            nc.sync.dma_start(qb_sb, qb)
            nc.sync.dma_start(swap_sb, swap)
            nc.sync.dma_start(ident_sb, ident)
            xth = [xh0, xh1]
        else:
            xth = []
            for half in range(2):
                xh = sb.tile([128, KT // 2, SC], BF16, tag="xt", name="xt",
                             bufs=6)
                nc.sync.dma_start(
                    xh, xT_r[:, half * (KT // 2):(half + 1) * (KT // 2), ssl])
                xth.append(xh)

        def xtile(t):
            return xth[t // (KT // 2)][:, t % (KT // 2), :]

        def rope(dst, ps, bias_col):
            """dst[:, ssl] = rope(ps + bias).

            raw = ps + bias (ACT, to SBUF); rot = swap_matrix @ raw (PE);
            dst = raw*cos + rot*sin_signed (DVE x2 + GpSimd add).
            """
            raw = sb.tile([128, SC], BF16, tag="qraw", name="qraw", bufs=4)
            nc.scalar.activation(raw, ps, AF.Identity, bias=bias_col)
            rot = psum.tile([128, SC], F32, tag="score", name="rotps", bufs=3)
            nc.tensor.matmul(rot, swap_sb, raw, start=True, stop=True)
            tmp = sb.tile([128, SC], BF16, tag="ropetmp", name="ropetmp",
                          bufs=2)
            nc.vector.tensor_mul(dst[:, ssl], raw, cost_sb[:, ssl])
            nc.vector.tensor_mul(tmp, rot, sint_sb[:, ssl])
            nc.gpsimd.tensor_add(dst[:, ssl], dst[:, ssl], tmp)

        # v first: its ACT-copy + PE-transpose chain overlaps the q matmuls
        ps = psum.tile([128, SC], F32, tag="av", name="proj", bufs=2)
        for t in range(KT):
            nc.tensor.matmul(ps, wkv_sb[:, t, DH:2 * DH], xtile(t),
                             start=(t == 0), stop=(t == KT - 1))
        vt = sb.tile([128, SC], BF16, tag="vt", name="vt", bufs=1)
        nc.scalar.activation(vt, ps, AF.Identity, bias=kvb_sb[:, 1:2])
        if sc == 0:
            load_rest_of_weights()

        # k
        ps = psum.tile([128, SC], F32, tag="av", name="proj", bufs=2)
        for t in range(KT):
            nc.tensor.matmul(ps, wkv_sb[:, t, 0:DH], xtile(t),
                             start=(t == 0), stop=(t == KT - 1))
        rope(k_rope, ps, kvb_sb[:, 0:1])

        for h in range(HPG):
            ps = psum.tile([128, SC], F32, tag="av", name="proj", bufs=2)
            for t in range(KT):
                nc.tensor.matmul(
                    ps, wq_sb[:, t, h * DH:(h + 1) * DH], xtile(t),
                    start=(t == 0), stop=(t == KT - 1))
            rope(q_rope[h], ps, qb_sb[:, h:h + 1])
            if h == 0:
                for j in range(SC // 128):
                    tp = psum.tile([128, 128], BF16, tag="tp", name="tpose",
                                   bufs=1)
                    nc.tensor.transpose(tp, vt[:, j * 128:(j + 1) * 128],
                                        ident_sb)
                    nc.scalar.activation(v_nat[:, sc * (SC // 128) + j, :],
                                         tp, AF.Copy)

    # ====== phases 2+3: causal attention with the o-projection slots
    # spread between attention regions, so the output DMA and PSUM-copy
    # work never bursts enough to stall the PE ======
    woT_r = woT.rearrange("(t p) n -> p t n", p=128)
    out_pr = out_p.rearrange("(sb p) n -> p sb n", p=128)

    # wo reuses the (dead after phase 1) wq slot: same tag, same size.
    wo = consts.tile([128, HPG, D], BF16, tag="wq", name="wo")
    for dc in range(4):
        dsl = slice(dc * 512, (dc + 1) * 512)
        nc.sync.dma_start(wo[:, :, dsl], woT_r[:, :, dsl])

    def emit_oproj_slot(qc, dc, pair, fine=False):
        """Project+store 2 s-blocks x 512 d-cols of q-chunk qc.

        Copies go on DVE only -- ACT is the attention phase's busiest
        engine (exp).  fine=True (trailing oproj) splits the output DMA
        per s-block to shorten the end-of-kernel drain.
        """
        dsl = slice(dc * 512, (dc + 1) * 512)
        osb = sb.tile([128, 2, 512], BF16, tag="osb", name="osb", bufs=4)
        for j in range(2):
            sblk = qc * 4 + pair * 2 + j
            op = psum.tile([128, 512], F32, tag="score", name="oproj",
                           bufs=3)
            for c in range(HPG):
                nc.tensor.matmul(
                    op, attn[c][:, sblk * 128:(sblk + 1) * 128],
                    wo[:, c, dsl], start=(c == 0), stop=(c == HPG - 1))
            nc.vector.tensor_copy(osb[:, j, :], op)
            if fine:
                nc.sync.dma_start(out_pr[:, sblk:sblk + 1, dsl],
                                  osb[:, j:j + 1, :])
        if not fine:
            nc.sync.dma_start(
                out_pr[:, qc * 4 + pair * 2:qc * 4 + pair * 2 + 2, dsl], osb)

    # Unit order interleaves qc=0 with qc=1 so the tiny qc=0 units (whose
    # normalization tail is longer than their PE time) hide inside big
    # neighbours; oproj(qc) slots then spread into the following units.
    units = ([(h, qc) for h in range(HPG) for qc in (0, 1)]
             + [(h, 2) for h in range(HPG)] + [(h, 3) for h in range(HPG)])
    seq = []
    for ui, (h, qc) in enumerate(units):
        for kj in range(4 * qc + 4):
            seq.append((ui, kj))
    ustate = {}
    done_h = {qc: 0 for qc in range(NQC)}

    def emit_scores(ui, kj):
        h, qc = units[ui]
        st = max(0, kj * 128 - qc * QC)
        width = QC - st
        sp = psum.tile([128, QC], F32, tag="score", name="score", bufs=3)
        nc.tensor.matmul(
            sp[:, 0:width],
            k_rope[:, kj * 128:(kj + 1) * 128],
            q_rope[h][:, qc * QC + st:(qc + 1) * QC],
            start=True, stop=True)
        if kj >= 4 * qc:  # region starts at the diagonal block
            nc.vector.tensor_add(sp[:, 0:128], sp[:, 0:128], mask_sb)
        et = sb.tile([128, QC], BF16, tag="exp", name="exp", bufs=4)
        nc.scalar.activation(et[:, 0:width], sp[:, 0:width], AF.Exp)
        return et, st, width

    def emit_av(ui, kj, ready):
        """Returns qc when this region completes all units of chunk qc."""
        h, qc = units[ui]
        et, st, width = ready
        if kj == 0:
            ustate[ui] = (
                psum.tile([128, QC], F32, tag="av", name="av", bufs=2),
                psum.tile([128, 4], F32, tag="sums", name="sums", bufs=2),
            )
        att_ps, sum_ps = ustate[ui]
        last = kj == 4 * qc + 3
        nc.tensor.matmul(
            att_ps[:, st:QC], v_nat[:, kj, :], et[:, 0:width],
            start=(kj == 0), stop=last, skip_group_check=True)
        # softmax sums, transposed: stationary = et q-block, moving = ones
        # -> out [q, 1]; PE cost ~1 cycle instead of `width`.  All 16ish
        # matmuls form ONE accumulation group on the bank: start only on
        # the very first (zeroes the whole bank buffer), stop only on the
        # very last (commits) -- an extra start would wipe pending sums.
        for qb in range(st // 128, 4):
            nc.tensor.matmul(
                sum_ps[:, qb:qb + 1],
                et[:, qb * 128 - st:qb * 128 - st + 128],
                onesk_sb,
                start=(kj == 0 and qb == 0),
                stop=(last and qb == 3),
                skip_group_check=True)
        if not last:
            return None
        rtmp = sb.tile([128, 4], BF16, tag="rtmp", name="rtmp", bufs=2)
        with nc.allow_low_precision(reason="bf16 softmax denom, ~0.4% ok"):
            nc.vector.reciprocal(rtmp, sum_ps)
        # transpose each 128-long reciprocal column to a [1, 128] row at
        # partition 0 (partition_broadcast requires a partition-0 source)
        tp = psum.tile([1, QC], BF16, tag="tp", name="rectp", bufs=1)
        for qb in range(4):
            nc.tensor.matmul(tp[0:1, qb * 128:(qb + 1) * 128],
                             rtmp[:, qb:qb + 1], ident_sb,
                             is_transpose=True,
                             start=(qb == 0), stop=(qb == 3),
                             skip_group_check=True)
        rrow = sb.tile([1, QC], BF16, tag="rrow", name="rrow", bufs=2)
        nc.scalar.activation(rrow, tp, AF.Copy)
        bcs = sb.tile([128, QC], BF16, tag="bcs", name="bcs", bufs=2)
        nc.gpsimd.partition_broadcast(bcs, rrow, channels=128)
        nc.vector.tensor_mul(attn[h][:, qc * QC:(qc + 1) * QC], att_ps, bcs)
        del ustate[ui]
        done_h[qc] += 1
        return qc if done_h[qc] == HPG else None

    pending = []
    LOOKAHEAD = 3
    ready = {}
    for i in range(min(LOOKAHEAD, len(seq))):
        ready[i] = emit_scores(*seq[i])
    for i in range(len(seq)):
        nxt = i + LOOKAHEAD
        if nxt < len(seq):
            ready[nxt] = emit_scores(*seq[nxt])
        fin = emit_av(*seq[i], ready.pop(i))
        if fin is not None:
            pending.extend(
                (fin, dc, pair) for dc in range(4) for pair in range(2))
        if pending and i % 2 == 0:
            emit_oproj_slot(*pending.pop(0))
    while pending:
        emit_oproj_slot(*pending.pop(0), fine=True)

    psum.release()
    sb.release()
    consts.release()


def _host_tables():
    c4 = np.float32(1.0) / np.sqrt(np.sqrt(np.float32(DH)))
    inv_freq = (np.float32(1.0) / np.power(
        np.float32(10000.0),
        np.arange(0, DH, 2, dtype=np.float32) / np.float32(DH))).astype(np.float32)
    t = np.arange(S, dtype=np.float32)
    freqs = np.outer(t, inv_freq).astype(np.float32)          # [S, 64]
    emb = np.concatenate([freqs, freqs], axis=1)              # [S, 128]
    cost = (np.cos(emb).T * c4).astype(NPBF16).copy()         # [128, S]
    sint = (np.sin(emb).T * c4).astype(np.float32)
    sint[0:64] *= np.float32(-1.0)                            # rotate_half sign
    sint = sint.astype(NPBF16).copy()
    kq = np.arange(128, dtype=np.int64)
    mask = np.where(kq[None, :] >= kq[:, None], np.float32(0.0),
                    np.float32(-1e9)).astype(np.float32)      # [k,q]
    ident = np.eye(128, dtype=np.float32).astype(NPBF16)
    # swap[i, j] = 1 iff j == (i+64) % 128; symmetric, so it works as lhsT.
    swap = np.zeros((128, 128), np.float32)
    swap[kq, (kq + 64) % 128] = np.float32(1.0)
    return cost, sint, mask, ident, swap.astype(NPBF16)


def kernel(x, q_weight, q_bias, kv_weight, kv_bias, o_weight, o_bias):
    x = np.asarray(x, np.float32)
    q_weight = np.asarray(q_weight, np.float32)
    q_bias = np.asarray(q_bias, np.float32)
    kv_weight = np.asarray(kv_weight, np.float32)
    kv_bias = np.asarray(kv_bias, np.float32)
    o_weight = np.asarray(o_weight, np.float32)
    o_bias = np.asarray(o_bias, np.float32)

    if "nc" not in _NC_CACHE:
        _NC_CACHE["nc"] = build_nc()
    nc = _NC_CACHE["nc"]

    cost, sint, mask, ident, swap = _host_tables()
    onesk = np.ones((128, 1), NPBF16)
    kvb = kv_bias.reshape(2, DH).T.copy()

    xTs = [np.ascontiguousarray(x[b].T).astype(NPBF16) for b in range(B)]
    wkvT = np.ascontiguousarray(kv_weight.T).astype(NPBF16)

    in_maps = []
    for core in range(8):
        b, g = divmod(core, G)
        in_maps.append({
            "xT": xTs[b],
            "wqT": np.ascontiguousarray(
                q_weight[g * C:(g + 1) * C].T).astype(NPBF16),
            "wkvT": wkvT,
            "woT": np.ascontiguousarray(
                o_weight[:, g * C:(g + 1) * C].T).astype(NPBF16),
            "qb": np.ascontiguousarray(
                q_bias[g * C:(g + 1) * C].reshape(HPG, DH).T),
            "kvb": kvb,
            "cost": cost,
            "sint": sint,
            "mask": mask,
            "ident": ident,
            "swap": swap,
            "onesk": onesk,
        })

    res = run_bass_kernel_spmd(nc, in_maps, core_ids=list(range(8)))

    out = np.zeros((B, S, D), np.float32)
    for core in range(8):
        out[core // G] += np.asarray(res.results[core]["out_p"],
                                     dtype=np.float32)
    out += o_bias[None, None, :]
    return out
